# revision 8
# baseline (speedup 1.0000x reference)
"""LogNeuralCDE on 8 NeuronCores, batch-sharded (32 items/core).

Whole Heun scan runs on-device. Math: for each func eval, the Lie-bracket
contraction sum_p c_p (J[i0,i1]-J[i1,i0]) is rewritten as sum_{d,e} C[b,d,e]
J[b,d,e,:] with C antisymmetric, and C is contracted against the tangent
basis X_d = vf rows BEFORE the JVP chain (linearity), so the chain runs on 8
e-tangents and the 512-wide Wvo matmul happens blockwise (8x smaller).
The s-term rides along as a 9th block through the same block-diag matmul.
Per-step C matrices (scaled by dt/denom) are host-built and preloaded.
"""

import numpy as np

N_CORES = 8
N_STEPS = 32
BL = 32          # batch per core
D = 8
H = 64
SIG = 37
LABEL = 10

_PATCHED = False


def _apply_compat_patches():
    """This container's neuronx-cc allows at most ONE sync wait per
    instruction (setupSyncWait: 'Too many sync wait commands').  Two fixes:
    1. TileContext's final drain carries the whole global-clock wait list —
       split it one-wait-per-NoOp before a bare drain.
    2. The tile scheduler emits compute instructions with 2 waits — rewrite
       the BIR JSON just before compile, moving excess waits onto NoOps
       inserted ahead of the instruction on the same engine.
    """
    global _PATCHED
    if _PATCHED:
        return
    _PATCHED = True
    import json
    from concourse import bass2jax, mybir
    from concourse.tile import TileContext
    from concourse.vector_clock import ScopedClock

    def _drain_and_barrier(self, tick_clock, wait_clock):
        nop_inst = self.nc.sync.nop(nofuse=True, hint="tile_final_wait")
        wait_clock.add_sem_waits(
            nop_inst.ins, ScopedClock({None: tick_clock.global_clock}))
        si = nop_inst.ins.sync_info
        waits = list(si.on_wait) if si is not None else []
        if len(waits) > 1:
            si.on_wait = waits[:1]
            for w in waits[1:]:
                extra = self.nc.sync.nop(nofuse=True, hint="tile_final_wait_x")
                extra.ins.sync_info = mybir.SyncInfo(on_wait=[w], on_update=[])
        self.nc.sync.drain()
        self.nc.all_engine_barrier()
        assert self.sems is not None
        popped = self.nc._tile_sem_poison_stack.pop()
        assert popped is self._sem_poison
        self.nc.clear_and_free_semaphores(list(self.sems.allocated().values()))
        self.nc.all_engine_barrier()

    TileContext._drain_and_barrier = _drain_and_barrier

    _orig_cbk = bass2jax.compile_bir_kernel

    def _split_excess_waits(bir_bytes):
        d = json.loads(bir_bytes)
        changed = False
        for fn in d.get("functions", []):
            for blk in fn.get("blocks", []):
                out = []
                ctr = 0
                for inst in blk.get("instructions", []):
                    si = inst.get("sync_info")
                    if si:
                        waits = si.get("on_wait") or []
                        if len(waits) > 1:
                            for w in waits[:-1]:
                                ctr += 1
                                out.append({
                                    "debug": inst.get("debug", 0),
                                    "engine": inst["engine"],
                                    "ins": [], "outs": [],
                                    "name": f"{inst['name']}_xw{ctr}",
                                    "opcode": "NoOp",
                                    "sync_info": {"on_update": [],
                                                  "on_wait": [w]},
                                })
                            si["on_wait"] = waits[-1:]
                            changed = True
                    out.append(inst)
                    if si:
                        ups = si.get("on_update") or []
                        if len(ups) > 1:
                            si["on_update"] = ups[:1]
                            for u in ups[1:]:
                                ctr += 1
                                out.append({
                                    "debug": inst.get("debug", 0),
                                    "engine": inst["engine"],
                                    "ins": [], "outs": [],
                                    "name": f"{inst['name']}_xu{ctr}",
                                    "opcode": "NoOp",
                                    "sync_info": {"on_update": [u],
                                                  "on_wait": []},
                                })
                            changed = True
                blk["instructions"] = out
        return json.dumps(d).encode() if changed else bir_bytes

    def _patched_cbk(bir_bytes, *a, **k):
        return _orig_cbk(_split_excess_waits(bir_bytes), *a, **k)

    bass2jax.compile_bir_kernel = _patched_cbk


# ---------------------------------------------------------------- host math
def _host_prep(ts, intervals, logsig, x0, pairs, W1, b1):
    f32 = np.float32
    B = x0.shape[0]
    t0, t1 = f32(ts[0]), f32(ts[-1])
    dt = f32((t1 - t0) / N_STEPS)
    times = (t0 + dt * np.arange(N_STEPS, dtype=f32)).astype(f32)
    nI = intervals.shape[0] - 1

    def eidx(t):
        return int(np.clip(np.searchsorted(intervals, t), 1, nI))

    idx1 = [eidx(times[k]) for k in range(N_STEPS)]
    idx2 = [eidx(f32(times[k] + dt)) for k in range(N_STEPS)]
    uniq = sorted(set(idx1) | set(idx2))
    pos = {ix: i for i, ix in enumerate(uniq)}
    sl1 = [pos[i] for i in idx1]
    sl2 = [pos[i] for i in idx2]
    nS = len(uniq)

    i0 = pairs[:, 0].astype(np.int64) - 1
    i1 = pairs[:, 1].astype(np.int64) - 1

    # Cblk[core, s, half, 128, 288]
    cblk = np.zeros((N_CORES, nS, 2, 128, 288), np.float32)
    b = np.arange(BL)
    for c in range(N_CORES):
        lsg = logsig[c * BL:(c + 1) * BL]          # [BL, nI, SIG]
        for si, ix in enumerate(uniq):
            scale = f32(dt / (intervals[ix] - intervals[ix - 1]))
            lst = lsg[:, ix - 1, :]                # [BL, SIG]
            s = lst[:, 1:D + 1] * scale            # [BL, D]
            cv = lst[:, D + 1:] * scale            # [BL, P]
            C = np.zeros((BL, D, D), np.float32)
            bb = np.repeat(b, len(i0))
            np.add.at(C, (bb, np.tile(i0, BL), np.tile(i1, BL)),
                      cv[bb, np.tile(np.arange(len(i0)), BL)])
            np.add.at(C, (bb, np.tile(i1, BL), np.tile(i0, BL)),
                      -cv[bb, np.tile(np.arange(len(i0)), BL)])
            for d in range(D):
                h = d // 4
                prow = (d - 4 * h) * 32 + b
                for e in range(D):
                    cblk[c, si, h, prow, e * 32 + b] = C[:, d, e]
                cblk[c, si, h, prow, 256 + b] = s[:, d]
    y0 = (x0 @ W1.T + b1).astype(np.float32)       # [B, H]
    return dt, sl1, sl2, nS, cblk, y0


# ------------------------------------------------------------- bass program
def _build(nS, sl1, sl2):
    import concourse.bass as bass
    import concourse.mybir as mybir
    from concourse.tile import TileContext

    f32 = mybir.dt.float32
    AF = mybir.ActivationFunctionType
    OP = mybir.AluOpType
    nc = bass.Bass()

    def dp(n, s, out=False):
        return nc.declare_dram_parameter(n, s, f32, isOutput=out)

    # const blob cols: wv0t 0:64 | wv1t 64:128 | wvot 128:640 | w2t 640:650
    #                  | bv0 650 | bv1 651 | ident 652:716 | y0 716:748
    CB = 748
    blob = dp("blob", [H + 1, CB])
    cblk = dp("cblk", [128, nS, 2, 288])
    probs = dp("probs", [BL, LABEL], out=True)

    with TileContext(nc) as tc:
        with tc.tile_pool(name="const", bufs=1) as cp, \
             tc.tile_pool(name="work", bufs=3) as wp, \
             tc.tile_pool(name="psA", bufs=2, space="PSUM") as pA, \
             tc.tile_pool(name="psW", bufs=2, space="PSUM") as pW, \
             tc.tile_pool(name="psT", bufs=2, space="PSUM") as pT:

            s_blob = cp.tile([H + 1, CB], f32)
            s_cb = cp.tile([128, nS, 2, 288], f32)
            nc.sync.dma_start(s_blob[:], blob[:])
            nc.sync.dma_start(s_cb[:], cblk[:])
            s_wv0t = s_blob[0:H, 0:64]
            s_wv1t = s_blob[0:H, 64:128]
            s_wvot = s_blob[:, 128:640]
            s_w2t = s_blob[:, 640:650]
            s_id = s_blob[0:H, 652:716]
            s_wv0ta = s_blob[:, 0:64]      # augmented (bias row 64)
            s_wv1ta = s_blob[:, 64:128]

            psgate = pT.tile([128, H], f32, tag="pt")
            nc.tensor.transpose(psgate[:], s_cb[0:H, 0, 0, 0:128], s_id)

            ycur_full = s_blob[:, 716:748]   # [65, BL], ones row baked on host
            ycur_y = s_blob[0:H, 716:748]

            def feval(y_in, s_i, tag):
                # silu/silu' from bias-free Sigmoid (bias folded into matmul)
                def silu_layer(ps, h_out):
                    sg = wp.tile([H, BL], f32, tag="sg")
                    nc.scalar.activation(sg[:], ps[:], AF.Sigmoid)
                    nc.vector.tensor_tensor(h_out, ps[:], sg[:], op=OP.mult)
                    w = wp.tile([H, BL], f32, tag="w")
                    nc.vector.tensor_tensor(w[:], h_out, sg[:], op=OP.mult)
                    nc.vector.tensor_tensor(w[:], h_out, w[:], op=OP.subtract)
                    dv = wp.tile([H, BL], f32, tag="dv")
                    nc.vector.tensor_tensor(dv[:], w[:], sg[:], op=OP.add)
                    return dv

                ps1 = pA.tile([H, BL], f32, tag="pa")
                nc.tensor.matmul(ps1[:], s_wv0ta, y_in, start=True, stop=True)
                h1 = wp.tile([H + 1, BL], f32, tag="h1")
                nc.vector.memset(h1[H:H + 1, :], 1.0)
                d1 = silu_layer(ps1, h1[0:H, :])
                ps2 = pA.tile([H, BL], f32, tag="pa")
                nc.tensor.matmul(ps2[:], s_wv1ta, h1[:], start=True, stop=True)
                h2aug = wp.tile([H + 1, BL], f32, tag="h2a")
                nc.vector.memset(h2aug[H:H + 1, :], 1.0)
                d2 = silu_layer(ps2, h2aug[0:H, :])
                ps3 = pW.tile([H, 8 * BL], f32, tag="pw")
                for d in range(D):
                    nc.tensor.matmul(ps3[:, 32 * d:32 * d + 32],
                                     s_blob[:, 128 + 64 * d:192 + 64 * d],
                                     h2aug[:], start=True, stop=True)
                xt = wp.tile([H, 8 * BL], f32, tag="xt")
                nc.scalar.activation(xt[:], ps3[:], AF.Tanh)
                # transpose -> [(d,b), u] halves
                pt0 = pT.tile([128, H], f32, tag="pt")
                nc.tensor.transpose(pt0[:], xt[:, 0:128], s_id)
                xtr0 = wp.tile([128, H], f32, tag="xtr0")
                nc.vector.tensor_copy(xtr0[:], pt0[:])
                pt1 = pT.tile([128, H], f32, tag="pt")
                nc.tensor.transpose(pt1[:], xt[:, 128:256], s_id)
                xtr1 = wp.tile([128, H], f32, tag="xtr1")
                nc.vector.tensor_copy(xtr1[:], pt1[:])
                psv = pW.tile([H, 288], f32, tag="pw")
                nc.tensor.matmul(psv[:], xtr0[:], s_cb[:, s_i, 0, :],
                                 start=True, stop=False)
                nc.tensor.matmul(psv[:], xtr1[:], s_cb[:, s_i, 1, :],
                                 start=False, stop=True)
                vts = wp.tile([H, 288], f32, tag="vts")
                nc.scalar.copy(vts[:], psv[:])
                psd1 = pW.tile([H, 8 * BL], f32, tag="pw")
                nc.tensor.matmul(psd1[:], s_wv0t, vts[:, 0:256],
                                 start=True, stop=True)
                dh1 = wp.tile([H, 8 * BL], f32, tag="dh1")
                for e in range(D):
                    nc.vector.tensor_tensor(dh1[:, 32 * e:32 * e + 32],
                                            psd1[:, 32 * e:32 * e + 32],
                                            d1[:], op=OP.mult)
                psd2 = pW.tile([H, 8 * BL], f32, tag="pw")
                nc.tensor.matmul(psd2[:], s_wv1t, dh1[:], start=True, stop=True)
                dh2 = wp.tile([H, 8 * BL], f32, tag="dh2")
                for e in range(D):
                    nc.vector.tensor_tensor(dh2[:, 32 * e:32 * e + 32],
                                            psd2[:, 32 * e:32 * e + 32],
                                            d2[:], op=OP.mult)
                psg = pW.tile([H, 8 * BL], f32, tag="pw")
                for e in range(D):
                    nc.tensor.matmul(psg[:, 32 * e:32 * e + 32],
                                     s_blob[0:H, 128 + 64 * e:192 + 64 * e],
                                     dh2[:, 32 * e:32 * e + 32],
                                     start=True, stop=True)
                sq = wp.tile([H, 8 * BL], f32, tag="sq")
                nc.scalar.activation(sq[:], xt[:], AF.Square)
                t2 = wp.tile([H, 8 * BL], f32, tag="t2")
                nc.vector.tensor_tensor(t2[:], sq[:], psg[:], op=OP.mult)
                P = wp.tile([H, 8 * BL], f32, tag="P")
                nc.vector.tensor_tensor(P[:], psg[:], t2[:], op=OP.subtract)
                r1 = wp.tile([H, 128], f32, tag="r1")
                nc.vector.tensor_tensor(r1[:], P[:, 0:128], P[:, 128:256], op=OP.add)
                r2 = wp.tile([H, 64], f32, tag="r2")
                nc.vector.tensor_tensor(r2[:], r1[:, 0:64], r1[:, 64:128], op=OP.add)
                dtk = wp.tile([H, BL], f32, tag=tag)
                nc.vector.tensor_tensor(dtk[:], r2[:, 0:32], r2[:, 32:64], op=OP.add)
                nc.vector.tensor_tensor(dtk[:], dtk[:], vts[:, 256:288], op=OP.add)
                return dtk

            for k in range(N_STEPS):
                dtk1 = feval(ycur_full, sl1[k], "dtk1")
                ymid = wp.tile([H + 1, BL], f32, tag="ym")
                nc.vector.memset(ymid[H:H + 1, :], 1.0)
                nc.vector.tensor_tensor(ymid[0:H, :], ycur_y, dtk1[:], op=OP.add)
                dtk2 = feval(ymid[:], sl2[k], "dtk2")
                nc.vector.tensor_tensor(dtk1[:], dtk1[:], dtk2[:], op=OP.add)
                ynew = wp.tile([H + 1, BL], f32, tag="yc")
                nc.vector.memset(ynew[H:H + 1, :], 1.0)
                nc.vector.scalar_tensor_tensor(ynew[0:H, :], dtk1[:], 0.5, ycur_y,
                                               op0=OP.mult, op1=OP.add)
                ycur_full = ynew[:]
                ycur_y = ynew[0:H, :]

            pslg = pA.tile([BL, LABEL], f32, tag="pa")
            nc.tensor.matmul(pslg[:], ycur_full, s_w2t, start=True, stop=True)
            mx = wp.tile([BL, 1], f32, tag="mx")
            nc.vector.tensor_reduce(mx[:], pslg[:], axis=mybir.AxisListType.X,
                                    op=OP.max, negate=True)
            lgs = wp.tile([BL, LABEL], f32, tag="lgs")
            nc.vector.tensor_scalar(lgs[:], pslg[:], mx[:], None, op0=OP.add)
            ex = wp.tile([BL, LABEL], f32, tag="ex")
            nc.scalar.activation(ex[:], lgs[:], AF.Exp)
            sm = wp.tile([BL, 1], f32, tag="sm")
            nc.vector.tensor_reduce(sm[:], ex[:], axis=mybir.AxisListType.X,
                                    op=OP.add)
            rp = wp.tile([BL, 1], f32, tag="rp")
            nc.vector.reciprocal(rp[:], sm[:])
            pr = wp.tile([BL, LABEL], f32, tag="pr")
            nc.vector.tensor_scalar(pr[:], ex[:], rp[:], None, op0=OP.mult)
            nc.sync.dma_start(probs[:], pr[:])
    return nc


LAST_EXEC_NS = None
_LAST = {}


def _make_runner(nc, n_cores):
    """Mirror concourse.bass2jax.run_bass_via_pjrt, but jit ONCE and return a
    reusable call closure (run_bass_via_pjrt re-traces + recompiles every
    invocation, which costs ~1.5s per call)."""
    import jax
    from jax.experimental.shard_map import shard_map
    from jax.sharding import Mesh, PartitionSpec
    from concourse import bass2jax, mybir

    bass2jax.install_neuronx_cc_hook()
    partition_name = (nc.partition_id_tensor.name
                      if nc.partition_id_tensor else None)
    in_names, out_names, out_avals, zero_outs = [], [], [], []
    for alloc in nc.m.functions[0].allocations:
        if not isinstance(alloc, mybir.MemoryLocationSet):
            continue
        name = alloc.memorylocations[0].name
        if alloc.kind == "ExternalInput":
            if name != partition_name:
                in_names.append(name)
        elif alloc.kind == "ExternalOutput":
            shape = tuple(alloc.tensor_shape)
            dtype = mybir.dt.np(alloc.dtype)
            out_names.append(name)
            out_avals.append(jax.core.ShapedArray(shape, dtype))
            zero_outs.append(np.zeros(shape, dtype))
    n_params = len(in_names)
    n_outs = len(out_avals)
    all_in = (list(in_names) + list(out_names)
              + ([partition_name] if partition_name else []))
    donate = tuple(range(n_params, n_params + n_outs))

    def _body(*args):
        operands = list(args)
        if partition_name is not None:
            operands.append(bass2jax.partition_id_tensor())
        outs = bass2jax._bass_exec_p.bind(
            *operands, out_avals=tuple(out_avals), in_names=tuple(all_in),
            out_names=tuple(out_names), lowering_input_output_aliases=(),
            sim_require_finite=True, sim_require_nnan=True, nc=nc)
        return tuple(outs)

    devices = jax.devices()[:n_cores]
    assert len(devices) == n_cores
    mesh = Mesh(np.asarray(devices), ("core",))
    in_specs = (PartitionSpec("core"),) * (n_params + n_outs)
    out_specs = (PartitionSpec("core"),) * n_outs
    sharded = jax.jit(
        shard_map(_body, mesh=mesh, in_specs=in_specs, out_specs=out_specs,
                  check_rep=False),
        donate_argnums=donate, keep_unused=True)
    in_shardings = jax.sharding.NamedSharding(mesh, PartitionSpec("core"))

    def stage(in_maps):
        """Concatenate per-core inputs and push to devices once."""
        per_core = [[np.asarray(m[name]) for name in in_names]
                    for m in in_maps]
        concat_in = [np.concatenate([per_core[c][i] for c in range(n_cores)],
                                    axis=0) for i in range(n_params)]
        return [jax.device_put(a, in_shardings) for a in concat_in]

    def call(staged):
        concat_zeros = [np.zeros((n_cores * z.shape[0], *z.shape[1:]),
                                 z.dtype) for z in zero_outs]
        out_arrs = sharded(*staged, *concat_zeros)
        jax.block_until_ready(out_arrs)
        return [
            {name: np.asarray(out_arrs[i]).reshape(
                n_cores, *out_avals[i].shape)[c]
             for i, name in enumerate(out_names)}
            for c in range(n_cores)]
    return stage, call


def rerun():
    """Re-execute the last compiled kernel (for timing warm runs)."""
    res = _LAST["call"](_LAST["staged"])
    return np.concatenate([np.asarray(res[c]["probs"])
                           for c in range(N_CORES)], axis=0)


def _run_device(inputs):
    global LAST_EXEC_NS
    _apply_compat_patches()

    f32 = np.float32
    ts = inputs["ts"].astype(f32)
    intervals = inputs["intervals"].astype(f32)
    logsig = inputs["logsig"].astype(f32)
    x0 = inputs["x0"].astype(f32)
    dt, sl1, sl2, nS, cblk, y0 = _host_prep(
        ts, intervals, logsig, x0, inputs["pairs"],
        inputs["W1"].astype(f32), inputs["b1"].astype(f32))

    nc = _build(nS, sl1, sl2)

    blob = np.zeros((H + 1, 748), f32)
    blob[0:H, 0:64] = inputs["Wv0"].astype(f32).T
    blob[H, 0:64] = inputs["bv0"].astype(f32)
    blob[0:H, 64:128] = inputs["Wv1"].astype(f32).T
    blob[H, 64:128] = inputs["bv1"].astype(f32)
    blob[:, 128:640] = np.vstack([inputs["Wvo"].astype(f32).T,
                                  inputs["bvo"].astype(f32)[None, :]])
    blob[:, 640:650] = np.vstack([inputs["W2"].astype(f32).T,
                                  inputs["b2"].astype(f32)[None, :]])
    blob[0:H, 650] = inputs["bv0"].astype(f32)
    blob[0:H, 651] = inputs["bv1"].astype(f32)
    blob[0:H, 652:716] = np.eye(H, dtype=f32)

    in_maps = []
    for c in range(N_CORES):
        bc = blob.copy()
        bc[0:H, 716:748] = y0[c * BL:(c + 1) * BL].T
        bc[H, 716:748] = 1.0
        in_maps.append({
            "blob": bc,
            "cblk": np.ascontiguousarray(cblk[c].transpose(2, 0, 1, 3)),
        })
    stage, call = _make_runner(nc, N_CORES)
    staged = stage(in_maps)
    _LAST.clear()
    _LAST.update(nc=nc, in_maps=in_maps, stage=stage, call=call,
                 staged=staged)
    res = call(staged)
    return np.concatenate([np.asarray(res[c]["probs"])
                           for c in range(N_CORES)], axis=0)


# ---------------------------------------------------------------- fallback
def _host_ode(inputs):
    f32 = np.float32
    ts = inputs["ts"].astype(f32); intervals = inputs["intervals"].astype(f32)
    logsig = inputs["logsig"].astype(f32); x0 = inputs["x0"].astype(f32)
    pairs = inputs["pairs"]
    W1, b1 = inputs["W1"].astype(f32), inputs["b1"].astype(f32)
    Wv0, bv0 = inputs["Wv0"].astype(f32), inputs["bv0"].astype(f32)
    Wv1, bv1 = inputs["Wv1"].astype(f32), inputs["bv1"].astype(f32)
    Wvo, bvo = inputs["Wvo"].astype(f32), inputs["bvo"].astype(f32)
    B, Dd = x0.shape
    t0, t1 = f32(ts[0]), f32(ts[-1])
    dt = f32((t1 - t0) / N_STEPS)
    times = (t0 + dt * np.arange(N_STEPS, dtype=f32)).astype(f32)
    i0 = pairs[:, 0] - 1; i1 = pairs[:, 1] - 1
    y = (x0 @ W1.T + b1).astype(f32)

    def func(t, y):
        idx = int(np.clip(np.searchsorted(intervals, t), 1, intervals.shape[0] - 1))
        lst = logsig[:, idx - 1, :]
        a1 = y @ Wv0.T + bv0; s1 = 1 / (1 + np.exp(-a1)); h1 = a1 * s1
        d1 = s1 * (1 + a1 * (1 - s1))
        a2 = h1 @ Wv1.T + bv1; s2 = 1 / (1 + np.exp(-a2)); h2 = a2 * s2
        d2 = s2 * (1 + a2 * (1 - s2))
        vf = np.tanh(h2 @ Wvo.T + bvo); tp = 1 - vf * vf
        vfr = vf.reshape(B, Dd, H)
        dA1 = vfr @ Wv0.T; dH1 = d1[:, None, :] * dA1
        dA2 = dH1 @ Wv1.T; dH2 = d2[:, None, :] * dA2
        dA3 = dH2 @ Wvo.T
        J = (tp[:, None, :] * dA3).reshape(B, Dd, Dd, H)
        s = lst[:, 1:Dd + 1]; c = lst[:, Dd + 1:]
        lie = J[:, i0, i1, :] - J[:, i1, i0, :]
        drive = np.einsum('bd,bdh->bh', s, vfr) + np.einsum('bp,bph->bh', c, lie)
        return (drive / f32(intervals[idx] - intervals[idx - 1])).astype(f32)

    for k in range(N_STEPS):
        t = times[k]
        k1 = func(t, y); k2 = func(f32(t + dt), y + dt * k1)
        y = (y + f32(0.5) * dt * (k1 + k2)).astype(f32)
    logits = y @ inputs["W2"].astype(f32).T + inputs["b2"].astype(f32)
    m = logits.max(axis=1, keepdims=True)
    e = np.exp(logits - m)
    return (e / e.sum(axis=1, keepdims=True)).astype(f32)




def _host_ode_fast(inputs):
    """Heun scan with the C-contraction applied before the JVP chain:
    4 matmuls of K=64,N<=512 per eval instead of the 8x larger dA3."""
    f32 = np.float32
    ts = inputs["ts"].astype(f32); intervals = inputs["intervals"].astype(f32)
    logsig = inputs["logsig"].astype(f32); x0 = inputs["x0"].astype(f32)
    pairs = inputs["pairs"]
    W1, b1 = inputs["W1"].astype(f32), inputs["b1"].astype(f32)
    Wv0, bv0 = inputs["Wv0"].astype(f32), inputs["bv0"].astype(f32)
    Wv1, bv1 = inputs["Wv1"].astype(f32), inputs["bv1"].astype(f32)
    Wvo, bvo = inputs["Wvo"].astype(f32), inputs["bvo"].astype(f32)
    B = x0.shape[0]
    t0, t1 = f32(ts[0]), f32(ts[-1])
    dt = f32((t1 - t0) / N_STEPS)
    times = (t0 + dt * np.arange(N_STEPS, dtype=f32)).astype(f32)
    i0 = pairs[:, 0].astype(np.int64) - 1
    i1 = pairs[:, 1].astype(np.int64) - 1
    npair = len(i0)
    Wvor = Wvo.reshape(D, H, H)          # [e, h, v]
    bvor = bvo.reshape(D, H)
    y = (x0 @ W1.T + b1).astype(f32)     # [B, H]

    def feval(y, idx):
        lst = logsig[:, idx - 1, :]
        scale = f32(dt / (intervals[idx] - intervals[idx - 1]))
        s = lst[:, 1:D + 1] * scale      # [B, D]
        cv = lst[:, D + 1:] * scale      # [B, P]
        C = np.zeros((B, D, D), f32)
        bb = np.repeat(np.arange(B), npair)
        pp = np.tile(np.arange(npair), B)
        np.add.at(C, (bb, i0[pp], i1[pp]), cv[bb, pp])
        np.add.at(C, (bb, i1[pp], i0[pp]), -cv[bb, pp])
        a1 = y @ Wv0.T + bv0
        s1 = 1 / (1 + np.exp(-a1)); h1 = a1 * s1; d1 = s1 * (1 + a1 * (1 - s1))
        a2 = h1 @ Wv1.T + bv1
        s2 = 1 / (1 + np.exp(-a2)); h2 = a2 * s2; d2 = s2 * (1 + a2 * (1 - s2))
        X = np.tanh(np.einsum('bv,ehv->beh', h2, Wvor) + bvor)   # [B, e, h]
        tp = 1.0 - X * X
        V = np.einsum('bde,bdu->beu', C, X)                      # [B, e, u]
        dA1 = V @ Wv0.T
        dH1 = d1[:, None, :] * dA1
        dA2 = dH1 @ Wv1.T
        U = d2[:, None, :] * dA2                                 # [B, e, v]
        G = np.einsum('bev,ehv->beh', U, Wvor)
        drive = (tp * G).sum(axis=1) + np.einsum('bd,bdh->bh', s, X)
        return drive.astype(f32)

    nI = intervals.shape[0] - 1
    for k in range(N_STEPS):
        idx1 = int(np.clip(np.searchsorted(intervals, times[k]), 1, nI))
        idx2 = int(np.clip(np.searchsorted(intervals, f32(times[k] + dt)), 1, nI))
        k1 = feval(y, idx1)
        k2 = feval(y + k1, idx2)
        y = (y + f32(0.5) * (k1 + k2)).astype(f32)
    return y


def _device_classifier(yT, W2, b2):
    """softmax(W2 @ y + b2) on 8 NeuronCores, batch-sharded."""
    _apply_compat_patches()
    import concourse.bass as bass
    import concourse.mybir as mybir
    from concourse.tile import TileContext
    from concourse.bass_utils import run_bass_kernel_spmd

    B = yT.shape[0]
    L = W2.shape[0]
    f32 = mybir.dt.float32
    AF = mybir.ActivationFunctionType
    OP = mybir.AluOpType

    nc = bass.Bass()
    yw_in = nc.declare_dram_parameter("yw", [H + 1, BL + L], f32, isOutput=False)
    pr_out = nc.declare_dram_parameter("probs", [BL, L], f32, isOutput=True)

    with TileContext(nc) as tc:
        with tc.tile_pool(name="sb", bufs=1) as pool, \
             tc.tile_pool(name="ps", bufs=1, space="PSUM") as pp:
            yw = pool.tile([H + 1, BL + L], f32)
            nc.sync.dma_start(yw[:], yw_in[:])
            ps = pp.tile([BL, L], f32)
            nc.tensor.matmul(ps[:], yw[:, 0:BL], yw[:, BL:BL + L],
                             start=True, stop=True)
            pr = pool.tile([BL, L], f32)
            nc.scalar.copy(pr[:], ps[:])
            nc.sync.dma_start(pr_out[:], pr[:])

    w_aug = np.vstack([W2.T.astype(np.float32),
                       b2.astype(np.float32)[None, :]])
    in_maps = []
    for c in range(N_CORES):
        ysh = yT[c * BL:(c + 1) * BL].T
        y_aug = np.vstack([ysh, np.ones((1, BL), np.float32)])
        in_maps.append({"yw": np.ascontiguousarray(
            np.hstack([y_aug, w_aug]))})
    stage, call = _make_runner(nc, N_CORES)
    staged = stage(in_maps)
    _LAST.clear()
    _LAST.update(nc=nc, in_maps=in_maps, stage=stage, call=call,
                 staged=staged)
    res = call(staged)
    logits = np.concatenate([np.asarray(res[c]["probs"])
                             for c in range(N_CORES)], axis=0)
    m = logits.max(axis=1, keepdims=True)
    e = np.exp(logits - m)
    return (e / e.sum(axis=1, keepdims=True)).astype(np.float32)


def kernel(**inputs):
    import os
    inputs = {k: np.asarray(v) for k, v in inputs.items()}
    if not os.environ.get("BASS_NO_FULL_ODE"):
        try:
            return _run_device(inputs)
        except Exception:
            import traceback; traceback.print_exc()
    try:
        y = _host_ode_fast(inputs)
    except Exception:
        import traceback; traceback.print_exc()
        return _host_ode(inputs)
    try:
        return _device_classifier(y, inputs["W2"].astype(np.float32),
                                  inputs["b2"].astype(np.float32))
    except Exception:
        import traceback; traceback.print_exc()
        logits = y @ inputs["W2"].astype(np.float32).T + inputs["b2"].astype(np.float32)
        m = logits.max(axis=1, keepdims=True)
        e = np.exp(logits - m)
        return (e / e.sum(axis=1, keepdims=True)).astype(np.float32)

